# revision 38
# baseline (speedup 1.0000x reference)
"""GraphWaveNet block kernel for 8 Trainium2 NeuronCores (v2).

Math (reference reduced; res_w branch is dead code):
  A = gcn_norm adjacency [N,N]; xa[m,t] = sum_n A[m,n] x[t,n]
  fg[o,m,t] = v0[o] xa[m,t] + v1[o] xa[m,t+1] + rowsum[m] bfg[o] + gcn_b[o]
  g [o,n,t] = p0[o] x[t,n]  + p1[o] x[t+1,n]  + bg[o]
  hg = tanh(fg) * sigmoid(g)
  out = end2 @ mean_t relu(end1 @ relu(skip @ hg + skb) + e1b) / TO + e2b

Design vs v1 baseline:
  - fg AND g for one t come out of a single K=66 fp16 matmul, channel-major
    (contract over a stacked rhs R = [xaT(32); x(32); rowsum; ones]), so no
    PE transposes and no broadcast elementwise ops.
  - all big GEMMs run in fp16 (PE full rate, half the SBUF read power of
    fp32r -> HAM stays at 2.4 GHz), PSUM accumulation stays fp32.
  - loop software-skewed: fg at t, skip at t-1, end1 at t-2, so the PE
    queue never waits on Act/DVE within a step.
  - the time-mean is folded into the end2 matmul: e2 @ r1_t accumulates
    in one PSUM bank across all 31 steps (one long accumulation group),
    so no separate mean pass exists at all.
  - relu+bias on Act (bias operand) for the skip conv and on DVE
    (tensor_scalar add+max) for end1; hg = tanh*sigmoid on GpSimd.

Sharding: data-parallel over batch, 1 batch element per core (B=8).
"""

import numpy as np

from concourse import bacc
from concourse import mybir
from concourse.bass_utils import run_bass_kernel_spmd
from concourse.tile import TileContext

FP = mybir.dt.float32
F16 = mybir.dt.float16

B, T, N, E = 8, 32, 512, 8192
TO = T - 1          # output time steps
RC = DC = 64
SC, EC, P = 256, 512, 12
NCORES = 8
NT = N // 128       # node tiles
KR = 66             # stacked-rhs contraction: 32 xaT + 32 x + rowsum + ones

# fp16 packed-constant layout: name -> free-dim width of the [128, w] segment
_SEGS16 = [
    ("xT", NT * T),            # lhsT tiles for phase A  [128 n, (kt, t)]
    ("at", NT * N),            # A^T tiles (rhs phase A) [128 n, (kt, m)]
    ("xnr", N),                # rows 0-31 x natural; row 32 rowsum; row 33 ones
    ("wt", TO * 128),          # rows 0-65: per-t fg/g lhsT matrices
    ("skt", SC),               # rows 0-63: skip_w^T
    ("e1t", (SC // 128) * EC),  # end1_w^T tiles [128, (kj, m)]
    ("e2t", (EC // 128) * P),   # end2_w^T tiles [128, (kj, u)]
]
_OFF16 = {}
_F16 = 0
for _nm, _w in _SEGS16:
    _OFF16[_nm] = _F16
    _F16 += _w

# fp32 packed-constant layout
_SEGS32 = [
    ("skb", SC // 128),
    ("e1b", EC // 128),
    ("e2b", 1),
]
_OFF32 = {}
_F32 = 0
for _nm, _w in _SEGS32:
    _OFF32[_nm] = _F32
    _F32 += _w


def _gcn_adj(edge_index, edge_weight, n):
    ei = np.asarray(edge_index)
    ew = np.asarray(edge_weight, dtype=np.float64)
    ar = np.arange(n)
    row = np.concatenate([ei[0], ar])
    col = np.concatenate([ei[1], ar])
    w = np.concatenate([ew, np.ones(n)])
    deg = np.zeros(n)
    np.add.at(deg, col, w)
    dis = np.where(deg > 0, 1.0 / np.sqrt(np.maximum(deg, 1e-300)), 0.0)
    norm = dis[row] * w * dis[col]
    A = np.zeros((n, n))
    np.add.at(A, (col, row), norm)
    return A  # A[tgt, src]


def _build_nc():
    nc = bacc.Bacc()
    d_h = nc.declare_dram_parameter("H", [128, _F16], F16, isOutput=False)
    d_f = nc.declare_dram_parameter("F", [128, _F32], FP, isOutput=False)
    d_out = nc.declare_dram_parameter("out", [P, N], FP, isOutput=True)

    AluOp = mybir.AluOpType
    Act = mybir.ActivationFunctionType

    o_xT = _OFF16["xT"]
    o_at = _OFF16["at"]
    o_xnr = _OFF16["xnr"]
    o_wt = _OFF16["wt"]
    o_skt = _OFF16["skt"]
    o_e1t = _OFF16["e1t"]
    o_e2t = _OFF16["e2t"]
    o_skb = _OFF32["skb"]
    o_e1b = _OFF32["e1b"]
    o_e2b = _OFF32["e2b"]

    with TileContext(nc) as tc:
        with (
            tc.tile_pool(name="consts", bufs=1) as consts,
            tc.tile_pool(name="work", bufs=2) as work,
            tc.tile_pool(name="ps", bufs=1, space="PSUM") as ps,
        ):
            ct = consts.tile([128, _F16], F16)
            cf = consts.tile([128, _F32], FP)
            R = consts.tile([KR, N], F16)
            # DMAs issued from different engines so the descriptor writes
            # don't serialize on one queue; transfers trimmed to the
            # partition rows actually used (xnr region is only read via
            # the R DMA).
            o_at1 = o_at + N
            nc.sync.dma_start(out=ct[:, :o_at1], in_=d_h[:, :o_at1])
            nc.sync.dma_start(out=ct[:, o_at1:o_xnr], in_=d_h[:, o_at1:o_xnr])
            nc.gpsimd.dma_start(
                out=ct[0:KR, o_wt:o_skt], in_=d_h[0:KR, o_wt:o_skt])
            nc.gpsimd.dma_start(
                out=ct[:, o_skt:o_e1t], in_=d_h[:, o_skt:o_e1t])
            # stacked rhs R: rows 0-31 xaT (computed), 32-63 x, 64 rowsum,
            # 65 ones
            nc.scalar.dma_start(out=R[32:KR], in_=d_h[0:34, o_xnr:o_xnr + N])
            nc.scalar.dma_start(out=ct[:, o_e1t:], in_=d_h[:, o_e1t:])
            nc.scalar.dma_start(out=cf[:], in_=d_f[:])

            # prime the one activation table set during the DMA wait
            prime = consts.tile([1, 1], FP)
            nc.vector.memset(prime[:], 0.0)
            nc.scalar.activation(prime[:], prime[:], Act.Sigmoid)

            # ---- phase A: xaT[t, m] = sum_n x[t, n] AT[n, m] ----
            xaps = ps.tile([32, N], FP, tag="pa", bufs=1)
            for kt in range(NT):
                nc.tensor.matmul(
                    xaps[:],
                    ct[:, o_xT + kt * T: o_xT + (kt + 1) * T],
                    ct[:, o_at + kt * N: o_at + (kt + 1) * N],
                    start=(kt == 0),
                    stop=(kt == NT - 1),
                )
            nc.scalar.copy(out=R[0:32], in_=xaps[:])

            # end2 accumulator: one PSUM bank, one accumulation group
            # spanning all (t, kj) — folds the time-mean into the matmul
            e2acc = ps.tile([P, N], FP, tag="pa", bufs=1)

            hgs = {}
            rsss = {}
            r1s = {}

            FGLEAD = 1  # fg stage runs this many t ahead of the skip stage

            def fg_stage(t):
                fgps = ps.tile([128, N], FP, tag="mm", bufs=7, name="fgps")
                nc.tensor.matmul(
                    fgps[:],
                    ct[0:KR, o_wt + t * 128: o_wt + (t + 1) * 128],
                    R[:],
                    start=True,
                    stop=True,
                )
                tnh = work.tile([64, N], F16, tag="tnh", bufs=FGLEAD + 2)
                sgm = work.tile([64, N], F16, tag="sgm", bufs=FGLEAD + 2)
                nc.scalar.activation(tnh[:], fgps[0:64], Act.Tanh)
                nc.scalar.activation(sgm[:], fgps[64:128], Act.Sigmoid)
                hg = work.tile([128, N], F16, tag="hg", bufs=FGLEAD + 2)
                nc.gpsimd.tensor_tensor(hg[0:64], tnh[:], sgm[:], AluOp.mult)
                nc.gpsimd.tensor_tensor(hg[64:128], tnh[:], sgm[:],
                                        AluOp.mult)
                hgs[t] = hg

            # warmup: run the fg stage ahead so the PE has dense work while
            # the Act->GpSimd hg chain fills
            for t in range(min(FGLEAD, TO)):
                fg_stage(t)

            # skewed main loop: fg at s+FGLEAD-1, skip at s-1, end1 at s-2,
            # end2 at s-3
            for s in range(1, TO + 3):
                t = s + FGLEAD - 1
                if t < TO:
                    fg_stage(t)

                tk = s - 1
                if 0 <= tk < TO:
                    hg = hgs.pop(tk)
                    sks = []
                    for sj in range(SC // 128):
                        skps = ps.tile([128, N], FP, tag="mm", bufs=7)
                        rb = 64 * sj
                        nc.tensor.matmul(
                            skps[:],
                            ct[rb:rb + 64,
                               o_skt + sj * 128: o_skt + (sj + 1) * 128],
                            hg[rb:rb + 64],
                            start=True,
                            stop=True,
                        )
                        sks.append(skps)
                    rss = []
                    for sj in range(SC // 128):
                        rs = work.tile([128, N], F16, tag=f"rss{sj}", bufs=4)
                        bias = cf[:, o_skb + sj: o_skb + sj + 1]
                        nc.scalar.activation(
                            rs[:], sks[sj][:], Act.Relu,
                            bias=bias, scale=1.0,
                        )
                        rss.append(rs)
                    rsss[tk] = rss

                te = s - 2
                t2 = s - 3

                def e1_mms(mj, rss):
                    e1ps = ps.tile([128, N], FP, tag="mm", bufs=7,
                                   name="e1ps")
                    for kj in range(SC // 128):
                        nc.tensor.matmul(
                            e1ps[:],
                            ct[:, o_e1t + kj * EC + mj * 128:
                               o_e1t + kj * EC + (mj + 1) * 128],
                            rss[kj][:],
                            start=(kj == 0),
                            stop=(kj == SC // 128 - 1),
                        )
                    return e1ps

                e1list = []
                if 0 <= te < TO:
                    rss = rsss.pop(te)
                    for mj in range(EC // 128):
                        e1list.append(e1_mms(mj, rss))

                # end2 accumulation for t = s-3 (one group over all t, kj)
                if 0 <= t2 < TO:
                    r1l = r1s.pop(t2)
                    for kj in range(EC // 128):
                        nc.tensor.matmul(
                            e2acc[:],
                            ct[:, o_e2t + kj * P: o_e2t + (kj + 1) * P],
                            r1l[kj][:],
                            start=(t2 == 0 and kj == 0),
                            stop=(t2 == TO - 1 and kj == EC // 128 - 1),
                        )

                if 0 <= te < TO:
                    r1l = []
                    for mj in range(EC // 128):
                        r1 = work.tile([128, N], F16, tag=f"r1_{mj}", bufs=3)
                        bias = cf[:, o_e1b + mj: o_e1b + mj + 1]
                        nc.vector.tensor_scalar(
                            r1[:], e1list[mj][:], bias, 0.0,
                            AluOp.add, AluOp.max,
                        )
                        r1l.append(r1)
                    r1s[te] = r1l

            # ---- scale 1/TO + bias on the accumulated end2 sum ----
            outsb = consts.tile([P, N], FP)
            nc.scalar.activation(
                outsb[:], e2acc[:], Act.Identity,
                bias=cf[0:P, o_e2b: o_e2b + 1], scale=1.0 / TO,
            )
            nc.sync.dma_start(out=d_out[:], in_=outsb[:])

    return nc


_NC_CACHE = {}


def _get_nc():
    if "nc" not in _NC_CACHE:
        nc = _build_nc()
        nc.finalize()
        _NC_CACHE["nc"] = nc
    return _NC_CACHE["nc"]


def kernel(x, edge_index, edge_weight, start_w, start_b, filt_w, filt_b,
           gate_w, gate_b, gcn_w, gcn_b, res_w, res_b, skip_w, skip_b,
           end1_w, end1_b, end2_w, end2_b, **_unused):
    x = np.asarray(x, dtype=np.float64)
    A = _gcn_adj(edge_index, edge_weight, N)          # float64 [tgt, src]
    rowsum = A.sum(axis=1)

    f64 = lambda a: np.asarray(a, dtype=np.float64)  # noqa: E731
    s = f64(start_w)[:, 0]
    sb = f64(start_b)
    fw, gw = f64(filt_w), f64(gate_w)
    gcn = f64(gcn_w)
    v0 = gcn @ (fw[:, :, 0] @ s)
    v1 = gcn @ (fw[:, :, 1] @ s)
    bfg = gcn @ ((fw[:, :, 0] + fw[:, :, 1]) @ sb + f64(filt_b))
    p0 = gw[:, :, 0] @ s
    p1 = gw[:, :, 1] @ s
    bgv = (gw[:, :, 0] + gw[:, :, 1]) @ sb + f64(gate_b)

    def part(a, ktiles):  # [(ktiles*128), M] -> [128, ktiles*M]
        a = np.asarray(a)
        return a.reshape(ktiles, 128, -1).transpose(1, 0, 2).reshape(128, -1)

    AT = np.ascontiguousarray(A.T)                     # [src n, tgt m]

    # per-t fg/g lhsT matrices, stacked along free dim
    wt = np.zeros((KR, TO * 128))
    for t in range(TO):
        c = t * 128
        wt[t, c:c + 64] = v0
        wt[t + 1, c:c + 64] = v1
        wt[64, c:c + 64] = bfg
        wt[65, c:c + 64] = f64(gcn_b)
        wt[32 + t, c + 64:c + 128] = p0
        wt[32 + t + 1, c + 64:c + 128] = p1
        wt[65, c + 64:c + 128] = bgv

    pack16 = np.zeros((128, _F16), dtype=np.float16)

    def put16(nm, arr):
        a = np.asarray(arr, dtype=np.float16)
        pack16[:a.shape[0], _OFF16[nm]:_OFF16[nm] + a.shape[1]] = a

    put16("at", part(AT, NT))
    put16("wt", wt)
    put16("skt", np.tile(f64(skip_w).T, (2, 1)))
    put16("e1t", part(f64(end1_w).T, SC // 128))
    put16("e2t", part(f64(end2_w).T, EC // 128))

    pack32 = np.zeros((128, _F32), dtype=np.float32)

    def put32(nm, arr):
        a = np.asarray(arr, dtype=np.float32)
        pack32[:a.shape[0], _OFF32[nm]:_OFF32[nm] + a.shape[1]] = a

    put32("skb", f64(skip_b).reshape(SC // 128, 128).T)
    put32("e1b", f64(end1_b).reshape(EC // 128, 128).T)
    put32("e2b", np.asarray(end2_b).reshape(P, 1))

    in_maps = []
    for b in range(B):
        pk = pack16.copy()
        xb = x[b]                                      # [T, N]
        pk[:, _OFF16["xT"]:_OFF16["xT"] + NT * T] = part(xb.T, NT)
        xnr = np.zeros((34, N))
        xnr[0:32] = xb
        xnr[32] = rowsum
        xnr[33] = 1.0
        pk[0:34, _OFF16["xnr"]:_OFF16["xnr"] + N] = xnr.astype(np.float16)
        in_maps.append({"H": pk, "F": pack32})

    nc = _get_nc()
    _NC_CACHE["in_maps"] = in_maps
    res = run_bass_kernel_spmd(nc, in_maps, list(range(NCORES)))
    out = np.stack([res.results[i]["out"] for i in range(B)])
    return out.astype(np.float32)                       # [B, P, N]


# revision 39
# speedup vs baseline: 1.0889x; 1.0889x over previous
"""GraphWaveNet block kernel for 8 Trainium2 NeuronCores (v2).

Math (reference reduced; res_w branch is dead code):
  A = gcn_norm adjacency [N,N]; xa[m,t] = sum_n A[m,n] x[t,n]
  fg[o,m,t] = v0[o] xa[m,t] + v1[o] xa[m,t+1] + rowsum[m] bfg[o] + gcn_b[o]
  g [o,n,t] = p0[o] x[t,n]  + p1[o] x[t+1,n]  + bg[o]
  hg = tanh(fg) * sigmoid(g)
  out = end2 @ mean_t relu(end1 @ relu(skip @ hg + skb) + e1b) / TO + e2b

Design vs v1 baseline:
  - fg AND g for one t come out of a single K=66 fp16 matmul, channel-major
    (contract over a stacked rhs R = [xaT(32); x(32); rowsum; ones]), so no
    PE transposes and no broadcast elementwise ops.
  - all big GEMMs run in fp16 (PE full rate, half the SBUF read power of
    fp32r -> HAM stays at 2.4 GHz), PSUM accumulation stays fp32.
  - loop software-skewed: fg at t, skip at t-1, end1 at t-2, so the PE
    queue never waits on Act/DVE within a step.
  - the time-mean is folded into the end2 matmul: e2 @ r1_t accumulates
    in one PSUM bank across all 31 steps (one long accumulation group),
    so no separate mean pass exists at all.
  - relu+bias on Act (bias operand) for the skip conv and on DVE
    (tensor_scalar add+max) for end1; hg = tanh*sigmoid on GpSimd.

Sharding: data-parallel over batch, 1 batch element per core (B=8).
"""

import numpy as np

from concourse import bacc
from concourse import mybir
from concourse.bass_utils import run_bass_kernel_spmd
from concourse.tile import TileContext

FP = mybir.dt.float32
F16 = mybir.dt.float16

B, T, N, E = 8, 32, 512, 8192
TO = T - 1          # output time steps
RC = DC = 64
SC, EC, P = 256, 512, 12
NCORES = 8
NT = N // 128       # node tiles
KR = 66             # stacked-rhs contraction: 32 xaT + 32 x + rowsum + ones

# fp16 packed-constant layout: name -> free-dim width of the [128, w] segment
_SEGS16 = [
    ("xT", NT * T),            # lhsT tiles for phase A  [128 n, (kt, t)]
    ("at", NT * N),            # A^T tiles (rhs phase A) [128 n, (kt, m)]
    ("xnr", N),                # rows 0-31 x natural; row 32 rowsum; row 33 ones
    ("wt", TO * 128),          # rows 0-65: per-t fg/g lhsT matrices
    ("skt", SC),               # rows 0-63: skip_w^T
    ("e1t", (SC // 128) * EC),  # end1_w^T tiles [128, (kj, m)]
    ("e2t", (EC // 128) * P),   # end2_w^T tiles [128, (kj, u)]
]
_OFF16 = {}
_F16 = 0
for _nm, _w in _SEGS16:
    _OFF16[_nm] = _F16
    _F16 += _w

# fp32 packed-constant layout
_SEGS32 = [
    ("skb", SC // 128),
    ("e1b", EC // 128),
    ("e2b", 1),
]
_OFF32 = {}
_F32 = 0
for _nm, _w in _SEGS32:
    _OFF32[_nm] = _F32
    _F32 += _w


def _gcn_adj(edge_index, edge_weight, n):
    ei = np.asarray(edge_index)
    ew = np.asarray(edge_weight, dtype=np.float64)
    ar = np.arange(n)
    row = np.concatenate([ei[0], ar])
    col = np.concatenate([ei[1], ar])
    w = np.concatenate([ew, np.ones(n)])
    deg = np.zeros(n)
    np.add.at(deg, col, w)
    dis = np.where(deg > 0, 1.0 / np.sqrt(np.maximum(deg, 1e-300)), 0.0)
    norm = dis[row] * w * dis[col]
    A = np.zeros((n, n))
    np.add.at(A, (col, row), norm)
    return A  # A[tgt, src]


def _build_nc():
    nc = bacc.Bacc()
    d_h = nc.declare_dram_parameter("H", [128, _F16], F16, isOutput=False)
    d_f = nc.declare_dram_parameter("F", [128, _F32], FP, isOutput=False)
    d_out = nc.declare_dram_parameter("out", [P, N], FP, isOutput=True)

    AluOp = mybir.AluOpType
    Act = mybir.ActivationFunctionType

    o_xT = _OFF16["xT"]
    o_at = _OFF16["at"]
    o_xnr = _OFF16["xnr"]
    o_wt = _OFF16["wt"]
    o_skt = _OFF16["skt"]
    o_e1t = _OFF16["e1t"]
    o_e2t = _OFF16["e2t"]
    o_skb = _OFF32["skb"]
    o_e1b = _OFF32["e1b"]
    o_e2b = _OFF32["e2b"]

    with TileContext(nc) as tc:
        with (
            tc.tile_pool(name="consts", bufs=1) as consts,
            tc.tile_pool(name="work", bufs=2) as work,
            tc.tile_pool(name="ps", bufs=1, space="PSUM") as ps,
        ):
            ct = consts.tile([128, _F16], F16)
            cf = consts.tile([128, _F32], FP)
            R = consts.tile([KR, N], F16)
            # DMAs issued from different engines so the descriptor writes
            # don't serialize on one queue; transfers trimmed to the
            # partition rows actually used (xnr region is only read via
            # the R DMA).
            o_at1 = o_at + N
            nc.sync.dma_start(out=ct[:, :o_at1], in_=d_h[:, :o_at1])
            nc.sync.dma_start(out=ct[:, o_at1:o_xnr], in_=d_h[:, o_at1:o_xnr])
            nc.gpsimd.dma_start(
                out=ct[0:KR, o_wt:o_skt], in_=d_h[0:KR, o_wt:o_skt])
            nc.gpsimd.dma_start(
                out=ct[:, o_skt:o_e1t], in_=d_h[:, o_skt:o_e1t])
            # stacked rhs R: rows 0-31 xaT (computed), 32-63 x, 64 rowsum,
            # 65 ones
            nc.scalar.dma_start(out=R[32:KR], in_=d_h[0:34, o_xnr:o_xnr + N])
            nc.scalar.dma_start(out=ct[:, o_e1t:], in_=d_h[:, o_e1t:])
            nc.scalar.dma_start(out=cf[:], in_=d_f[:])

            # prime the one activation table set during the DMA wait
            prime = consts.tile([1, 1], FP)
            nc.vector.memset(prime[:], 0.0)
            nc.scalar.activation(prime[:], prime[:], Act.Sigmoid)

            # ---- phase A: xaT[t, m] = sum_n x[t, n] AT[n, m] ----
            xaps = ps.tile([32, N], FP, tag="pa", bufs=1)
            for kt in range(NT):
                nc.tensor.matmul(
                    xaps[:],
                    ct[:, o_xT + kt * T: o_xT + (kt + 1) * T],
                    ct[:, o_at + kt * N: o_at + (kt + 1) * N],
                    start=(kt == 0),
                    stop=(kt == NT - 1),
                )
            nc.scalar.copy(out=R[0:32], in_=xaps[:])

            # end2 accumulator: one PSUM bank, one accumulation group
            # spanning all (t, kj) — folds the time-mean into the matmul
            e2acc = ps.tile([P, N], FP, tag="pa", bufs=1)

            hgs = {}
            rsss = {}
            r1s = {}

            FGLEAD = 1  # fg stage runs this many t ahead of the skip stage

            def fg_stage(t):
                fgps = ps.tile([128, N], FP, tag="mm", bufs=7, name="fgps")
                nc.tensor.matmul(
                    fgps[:],
                    ct[0:KR, o_wt + t * 128: o_wt + (t + 1) * 128],
                    R[:],
                    start=True,
                    stop=True,
                )
                tnh = work.tile([64, N], F16, tag="tnh", bufs=FGLEAD + 2)
                sgm = work.tile([64, N], F16, tag="sgm", bufs=FGLEAD + 2)
                nc.scalar.activation(tnh[:], fgps[0:64], Act.Tanh)
                nc.scalar.activation(sgm[:], fgps[64:128], Act.Sigmoid)
                hg = work.tile([64, N], F16, tag="hg", bufs=FGLEAD + 2)
                nc.gpsimd.tensor_tensor(hg[:], tnh[:], sgm[:], AluOp.mult)
                hgs[t] = hg

            # warmup: run the fg stage ahead so the PE has dense work while
            # the Act->GpSimd hg chain fills
            for t in range(min(FGLEAD, TO)):
                fg_stage(t)

            # skewed main loop: fg at s+FGLEAD-1, skip at s-1, end1 at s-2,
            # end2 at s-3
            for s in range(1, TO + 3):
                t = s + FGLEAD - 1
                if t < TO:
                    fg_stage(t)

                tk = s - 1
                if 0 <= tk < TO:
                    hg = hgs.pop(tk)
                    sks = []
                    for sj in range(SC // 128):
                        skps = ps.tile([128, N], FP, tag="mm", bufs=7)
                        nc.tensor.matmul(
                            skps[:],
                            ct[0:64, o_skt + sj * 128: o_skt + (sj + 1) * 128],
                            hg[0:64],
                            start=True,
                            stop=True,
                        )
                        sks.append(skps)
                    rss = []
                    for sj in range(SC // 128):
                        rs = work.tile([128, N], F16, tag=f"rss{sj}", bufs=4)
                        bias = cf[:, o_skb + sj: o_skb + sj + 1]
                        nc.scalar.activation(
                            rs[:], sks[sj][:], Act.Relu,
                            bias=bias, scale=1.0,
                        )
                        rss.append(rs)
                    rsss[tk] = rss

                te = s - 2
                t2 = s - 3

                def e1_mms(mj, rss):
                    e1ps = ps.tile([128, N], FP, tag="mm", bufs=7,
                                   name="e1ps")
                    for kj in range(SC // 128):
                        nc.tensor.matmul(
                            e1ps[:],
                            ct[:, o_e1t + kj * EC + mj * 128:
                               o_e1t + kj * EC + (mj + 1) * 128],
                            rss[kj][:],
                            start=(kj == 0),
                            stop=(kj == SC // 128 - 1),
                        )
                    return e1ps

                e1list = []
                if 0 <= te < TO:
                    rss = rsss.pop(te)
                    for mj in range(EC // 128):
                        e1list.append(e1_mms(mj, rss))

                # end2 accumulation for t = s-3 (one group over all t, kj)
                if 0 <= t2 < TO:
                    r1l = r1s.pop(t2)
                    for kj in range(EC // 128):
                        nc.tensor.matmul(
                            e2acc[:],
                            ct[:, o_e2t + kj * P: o_e2t + (kj + 1) * P],
                            r1l[kj][:],
                            start=(t2 == 0 and kj == 0),
                            stop=(t2 == TO - 1 and kj == EC // 128 - 1),
                        )

                if 0 <= te < TO:
                    r1l = []
                    for mj in range(EC // 128):
                        r1 = work.tile([128, N], F16, tag=f"r1_{mj}", bufs=3)
                        bias = cf[:, o_e1b + mj: o_e1b + mj + 1]
                        nc.vector.tensor_scalar(
                            r1[:], e1list[mj][:], bias, 0.0,
                            AluOp.add, AluOp.max,
                        )
                        r1l.append(r1)
                    r1s[te] = r1l

            # ---- scale 1/TO + bias on the accumulated end2 sum ----
            outsb = consts.tile([P, N], FP)
            nc.scalar.activation(
                outsb[:], e2acc[:], Act.Identity,
                bias=cf[0:P, o_e2b: o_e2b + 1], scale=1.0 / TO,
            )
            nc.sync.dma_start(out=d_out[:], in_=outsb[:])

    return nc


_NC_CACHE = {}


def _get_nc():
    if "nc" not in _NC_CACHE:
        nc = _build_nc()
        nc.finalize()
        _NC_CACHE["nc"] = nc
    return _NC_CACHE["nc"]


def kernel(x, edge_index, edge_weight, start_w, start_b, filt_w, filt_b,
           gate_w, gate_b, gcn_w, gcn_b, res_w, res_b, skip_w, skip_b,
           end1_w, end1_b, end2_w, end2_b, **_unused):
    x = np.asarray(x, dtype=np.float64)
    A = _gcn_adj(edge_index, edge_weight, N)          # float64 [tgt, src]
    rowsum = A.sum(axis=1)

    f64 = lambda a: np.asarray(a, dtype=np.float64)  # noqa: E731
    s = f64(start_w)[:, 0]
    sb = f64(start_b)
    fw, gw = f64(filt_w), f64(gate_w)
    gcn = f64(gcn_w)
    v0 = gcn @ (fw[:, :, 0] @ s)
    v1 = gcn @ (fw[:, :, 1] @ s)
    bfg = gcn @ ((fw[:, :, 0] + fw[:, :, 1]) @ sb + f64(filt_b))
    p0 = gw[:, :, 0] @ s
    p1 = gw[:, :, 1] @ s
    bgv = (gw[:, :, 0] + gw[:, :, 1]) @ sb + f64(gate_b)

    def part(a, ktiles):  # [(ktiles*128), M] -> [128, ktiles*M]
        a = np.asarray(a)
        return a.reshape(ktiles, 128, -1).transpose(1, 0, 2).reshape(128, -1)

    AT = np.ascontiguousarray(A.T)                     # [src n, tgt m]

    # per-t fg/g lhsT matrices, stacked along free dim
    wt = np.zeros((KR, TO * 128))
    for t in range(TO):
        c = t * 128
        wt[t, c:c + 64] = v0
        wt[t + 1, c:c + 64] = v1
        wt[64, c:c + 64] = bfg
        wt[65, c:c + 64] = f64(gcn_b)
        wt[32 + t, c + 64:c + 128] = p0
        wt[32 + t + 1, c + 64:c + 128] = p1
        wt[65, c + 64:c + 128] = bgv

    pack16 = np.zeros((128, _F16), dtype=np.float16)

    def put16(nm, arr):
        a = np.asarray(arr, dtype=np.float16)
        pack16[:a.shape[0], _OFF16[nm]:_OFF16[nm] + a.shape[1]] = a

    put16("at", part(AT, NT))
    put16("wt", wt)
    put16("skt", np.tile(f64(skip_w).T, (2, 1)))
    put16("e1t", part(f64(end1_w).T, SC // 128))
    put16("e2t", part(f64(end2_w).T, EC // 128))

    pack32 = np.zeros((128, _F32), dtype=np.float32)

    def put32(nm, arr):
        a = np.asarray(arr, dtype=np.float32)
        pack32[:a.shape[0], _OFF32[nm]:_OFF32[nm] + a.shape[1]] = a

    put32("skb", f64(skip_b).reshape(SC // 128, 128).T)
    put32("e1b", f64(end1_b).reshape(EC // 128, 128).T)
    put32("e2b", np.asarray(end2_b).reshape(P, 1))

    in_maps = []
    for b in range(B):
        pk = pack16.copy()
        xb = x[b]                                      # [T, N]
        pk[:, _OFF16["xT"]:_OFF16["xT"] + NT * T] = part(xb.T, NT)
        xnr = np.zeros((34, N))
        xnr[0:32] = xb
        xnr[32] = rowsum
        xnr[33] = 1.0
        pk[0:34, _OFF16["xnr"]:_OFF16["xnr"] + N] = xnr.astype(np.float16)
        in_maps.append({"H": pk, "F": pack32})

    nc = _get_nc()
    _NC_CACHE["in_maps"] = in_maps
    res = run_bass_kernel_spmd(nc, in_maps, list(range(NCORES)))
    out = np.stack([res.results[i]["out"] for i in range(B)])
    return out.astype(np.float32)                       # [B, P, N]


# revision 40
# speedup vs baseline: 1.0926x; 1.0034x over previous
"""GraphWaveNet block kernel for 8 Trainium2 NeuronCores (v2).

Math (reference reduced; res_w branch is dead code):
  A = gcn_norm adjacency [N,N]; xa[m,t] = sum_n A[m,n] x[t,n]
  fg[o,m,t] = v0[o] xa[m,t] + v1[o] xa[m,t+1] + rowsum[m] bfg[o] + gcn_b[o]
  g [o,n,t] = p0[o] x[t,n]  + p1[o] x[t+1,n]  + bg[o]
  hg = tanh(fg) * sigmoid(g)
  out = end2 @ mean_t relu(end1 @ relu(skip @ hg + skb) + e1b) / TO + e2b

Design vs v1 baseline:
  - fg AND g for one t come out of a single K=66 fp16 matmul, channel-major
    (contract over a stacked rhs R = [xaT(32); x(32); rowsum; ones]), so no
    PE transposes and no broadcast elementwise ops.
  - all big GEMMs run in fp16 (PE full rate, half the SBUF read power of
    fp32r -> HAM stays at 2.4 GHz), PSUM accumulation stays fp32.
  - loop software-skewed: fg at t, skip at t-1, end1 at t-2, so the PE
    queue never waits on Act/DVE within a step.
  - the time-mean is folded into the end2 matmul: e2 @ r1_t accumulates
    in one PSUM bank across all 31 steps (one long accumulation group),
    so no separate mean pass exists at all.
  - relu+bias on Act (bias operand) for the skip conv and on DVE
    (tensor_scalar add+max) for end1; hg = tanh*sigmoid on GpSimd.

Sharding: data-parallel over batch, 1 batch element per core (B=8).
"""

import numpy as np

from concourse import bacc
from concourse import mybir
from concourse.bass_utils import run_bass_kernel_spmd
from concourse.tile import TileContext

FP = mybir.dt.float32
F16 = mybir.dt.float16

B, T, N, E = 8, 32, 512, 8192
TO = T - 1          # output time steps
RC = DC = 64
SC, EC, P = 256, 512, 12
NCORES = 8
NT = N // 128       # node tiles
KR = 66             # stacked-rhs contraction: 32 xaT + 32 x + rowsum + ones

# fp16 packed-constant layout: name -> free-dim width of the [128, w] segment
_SEGS16 = [
    ("xT", NT * T),            # lhsT tiles for phase A  [128 n, (kt, t)]
    ("at", NT * N),            # A^T tiles (rhs phase A) [128 n, (kt, m)]
    ("xnr", N),                # rows 0-31 x natural; row 32 rowsum; row 33 ones
    ("wt", TO * 128),          # rows 0-65: per-t fg/g lhsT matrices
    ("skt", SC),               # rows 0-63: skip_w^T
    ("e1t", (SC // 128) * EC),  # end1_w^T tiles [128, (kj, m)]
    ("e2t", (EC // 128) * P),   # end2_w^T tiles [128, (kj, u)]
]
_OFF16 = {}
_F16 = 0
for _nm, _w in _SEGS16:
    _OFF16[_nm] = _F16
    _F16 += _w

# fp32 packed-constant layout
_SEGS32 = [
    ("skb", SC // 128),
    ("e1b", EC // 128),
    ("e2b", 1),
]
_OFF32 = {}
_F32 = 0
for _nm, _w in _SEGS32:
    _OFF32[_nm] = _F32
    _F32 += _w


def _gcn_adj(edge_index, edge_weight, n):
    ei = np.asarray(edge_index)
    ew = np.asarray(edge_weight, dtype=np.float64)
    ar = np.arange(n)
    row = np.concatenate([ei[0], ar])
    col = np.concatenate([ei[1], ar])
    w = np.concatenate([ew, np.ones(n)])
    deg = np.zeros(n)
    np.add.at(deg, col, w)
    dis = np.where(deg > 0, 1.0 / np.sqrt(np.maximum(deg, 1e-300)), 0.0)
    norm = dis[row] * w * dis[col]
    A = np.zeros((n, n))
    np.add.at(A, (col, row), norm)
    return A  # A[tgt, src]


def _build_nc():
    nc = bacc.Bacc()
    d_h = nc.declare_dram_parameter("H", [128, _F16], F16, isOutput=False)
    d_f = nc.declare_dram_parameter("F", [128, _F32], FP, isOutput=False)
    d_out = nc.declare_dram_parameter("out", [P, N], FP, isOutput=True)

    AluOp = mybir.AluOpType
    Act = mybir.ActivationFunctionType

    o_xT = _OFF16["xT"]
    o_at = _OFF16["at"]
    o_xnr = _OFF16["xnr"]
    o_wt = _OFF16["wt"]
    o_skt = _OFF16["skt"]
    o_e1t = _OFF16["e1t"]
    o_e2t = _OFF16["e2t"]
    o_skb = _OFF32["skb"]
    o_e1b = _OFF32["e1b"]
    o_e2b = _OFF32["e2b"]

    with TileContext(nc) as tc:
        with (
            tc.tile_pool(name="consts", bufs=1) as consts,
            tc.tile_pool(name="work", bufs=2) as work,
            tc.tile_pool(name="ps", bufs=1, space="PSUM") as ps,
        ):
            ct = consts.tile([128, _F16], F16)
            cf = consts.tile([128, _F32], FP)
            R = consts.tile([KR, N], F16)
            # DMAs issued from different engines so the descriptor writes
            # don't serialize on one queue; transfers trimmed to the
            # partition rows actually used (xnr region is only read via
            # the R DMA).
            o_at1 = o_at + N
            nc.sync.dma_start(out=ct[:, :o_at1], in_=d_h[:, :o_at1])
            nc.sync.dma_start(out=ct[:, o_at1:o_xnr], in_=d_h[:, o_at1:o_xnr])
            nc.gpsimd.dma_start(
                out=ct[0:KR, o_wt:o_skt], in_=d_h[0:KR, o_wt:o_skt])
            nc.gpsimd.dma_start(
                out=ct[0:64, o_skt:o_e1t], in_=d_h[0:64, o_skt:o_e1t])
            # stacked rhs R: rows 0-31 xaT (computed), 32-63 x, 64 rowsum,
            # 65 ones
            nc.scalar.dma_start(out=R[32:KR], in_=d_h[0:34, o_xnr:o_xnr + N])
            nc.scalar.dma_start(out=ct[:, o_e1t:], in_=d_h[:, o_e1t:])
            nc.scalar.dma_start(out=cf[:], in_=d_f[:])

            # prime the one activation table set during the DMA wait
            prime = consts.tile([1, 1], FP)
            nc.vector.memset(prime[:], 0.0)
            nc.scalar.activation(prime[:], prime[:], Act.Sigmoid)

            # ---- phase A: xaT[t, m] = sum_n x[t, n] AT[n, m] ----
            xaps = ps.tile([32, N], FP, tag="pa", bufs=1)
            for kt in range(NT):
                nc.tensor.matmul(
                    xaps[:],
                    ct[:, o_xT + kt * T: o_xT + (kt + 1) * T],
                    ct[:, o_at + kt * N: o_at + (kt + 1) * N],
                    start=(kt == 0),
                    stop=(kt == NT - 1),
                )
            nc.scalar.copy(out=R[0:32], in_=xaps[:])

            # end2 accumulator: one PSUM bank, one accumulation group
            # spanning all (t, kj) — folds the time-mean into the matmul
            e2acc = ps.tile([P, N], FP, tag="pa", bufs=1)

            hgs = {}
            rsss = {}
            r1s = {}

            FGLEAD = 1  # fg stage runs this many t ahead of the skip stage

            def fg_stage(t):
                fgps = ps.tile([128, N], FP, tag="mm", bufs=7, name="fgps")
                nc.tensor.matmul(
                    fgps[:],
                    ct[0:KR, o_wt + t * 128: o_wt + (t + 1) * 128],
                    R[:],
                    start=True,
                    stop=True,
                )
                tnh = work.tile([64, N], F16, tag="tnh", bufs=FGLEAD + 2)
                sgm = work.tile([64, N], F16, tag="sgm", bufs=FGLEAD + 2)
                nc.scalar.activation(tnh[:], fgps[0:64], Act.Tanh)
                nc.scalar.activation(sgm[:], fgps[64:128], Act.Sigmoid)
                hg = work.tile([64, N], F16, tag="hg", bufs=FGLEAD + 2)
                nc.gpsimd.tensor_tensor(hg[:], tnh[:], sgm[:], AluOp.mult)
                hgs[t] = hg

            # warmup: run the fg stage ahead so the PE has dense work while
            # the Act->GpSimd hg chain fills
            for t in range(min(FGLEAD, TO)):
                fg_stage(t)

            # skewed main loop: fg at s+FGLEAD-1, skip at s-1, end1 at s-2,
            # end2 at s-3
            for s in range(1, TO + 3):
                t = s + FGLEAD - 1
                if t < TO:
                    fg_stage(t)

                tk = s - 1
                if 0 <= tk < TO:
                    hg = hgs.pop(tk)
                    sks = []
                    for sj in range(SC // 128):
                        skps = ps.tile([128, N], FP, tag="mm", bufs=7)
                        nc.tensor.matmul(
                            skps[:],
                            ct[0:64, o_skt + sj * 128: o_skt + (sj + 1) * 128],
                            hg[0:64],
                            start=True,
                            stop=True,
                        )
                        sks.append(skps)
                    rss = []
                    for sj in range(SC // 128):
                        rs = work.tile([128, N], F16, tag=f"rss{sj}", bufs=4)
                        bias = cf[:, o_skb + sj: o_skb + sj + 1]
                        nc.scalar.activation(
                            rs[:], sks[sj][:], Act.Relu,
                            bias=bias, scale=1.0,
                        )
                        rss.append(rs)
                    rsss[tk] = rss

                te = s - 2
                t2 = s - 3

                def e1_mms(mj, rss):
                    e1ps = ps.tile([128, N], FP, tag="mm", bufs=7,
                                   name="e1ps")
                    for kj in range(SC // 128):
                        nc.tensor.matmul(
                            e1ps[:],
                            ct[:, o_e1t + kj * EC + mj * 128:
                               o_e1t + kj * EC + (mj + 1) * 128],
                            rss[kj][:],
                            start=(kj == 0),
                            stop=(kj == SC // 128 - 1),
                        )
                    return e1ps

                e1list = []
                if 0 <= te < TO:
                    rss = rsss.pop(te)
                    for mj in range(EC // 128):
                        e1list.append(e1_mms(mj, rss))

                # end2 accumulation for t = s-3 (one group over all t, kj)
                if 0 <= t2 < TO:
                    r1l = r1s.pop(t2)
                    for kj in range(EC // 128):
                        nc.tensor.matmul(
                            e2acc[:],
                            ct[:, o_e2t + kj * P: o_e2t + (kj + 1) * P],
                            r1l[kj][:],
                            start=(t2 == 0 and kj == 0),
                            stop=(t2 == TO - 1 and kj == EC // 128 - 1),
                        )

                if 0 <= te < TO:
                    r1l = []
                    for mj in range(EC // 128):
                        r1 = work.tile([128, N], F16, tag=f"r1_{mj}", bufs=3)
                        bias = cf[:, o_e1b + mj: o_e1b + mj + 1]
                        nc.vector.tensor_scalar(
                            r1[:], e1list[mj][:], bias, 0.0,
                            AluOp.add, AluOp.max,
                        )
                        r1l.append(r1)
                    r1s[te] = r1l

            # ---- scale 1/TO + bias on the accumulated end2 sum ----
            outsb = consts.tile([P, N], FP)
            nc.scalar.activation(
                outsb[:], e2acc[:], Act.Identity,
                bias=cf[0:P, o_e2b: o_e2b + 1], scale=1.0 / TO,
            )
            nc.sync.dma_start(out=d_out[:], in_=outsb[:])

    return nc


_NC_CACHE = {}


def _get_nc():
    if "nc" not in _NC_CACHE:
        nc = _build_nc()
        nc.finalize()
        _NC_CACHE["nc"] = nc
    return _NC_CACHE["nc"]


def kernel(x, edge_index, edge_weight, start_w, start_b, filt_w, filt_b,
           gate_w, gate_b, gcn_w, gcn_b, res_w, res_b, skip_w, skip_b,
           end1_w, end1_b, end2_w, end2_b, **_unused):
    x = np.asarray(x, dtype=np.float64)
    A = _gcn_adj(edge_index, edge_weight, N)          # float64 [tgt, src]
    rowsum = A.sum(axis=1)

    f64 = lambda a: np.asarray(a, dtype=np.float64)  # noqa: E731
    s = f64(start_w)[:, 0]
    sb = f64(start_b)
    fw, gw = f64(filt_w), f64(gate_w)
    gcn = f64(gcn_w)
    v0 = gcn @ (fw[:, :, 0] @ s)
    v1 = gcn @ (fw[:, :, 1] @ s)
    bfg = gcn @ ((fw[:, :, 0] + fw[:, :, 1]) @ sb + f64(filt_b))
    p0 = gw[:, :, 0] @ s
    p1 = gw[:, :, 1] @ s
    bgv = (gw[:, :, 0] + gw[:, :, 1]) @ sb + f64(gate_b)

    def part(a, ktiles):  # [(ktiles*128), M] -> [128, ktiles*M]
        a = np.asarray(a)
        return a.reshape(ktiles, 128, -1).transpose(1, 0, 2).reshape(128, -1)

    AT = np.ascontiguousarray(A.T)                     # [src n, tgt m]

    # per-t fg/g lhsT matrices, stacked along free dim
    wt = np.zeros((KR, TO * 128))
    for t in range(TO):
        c = t * 128
        wt[t, c:c + 64] = v0
        wt[t + 1, c:c + 64] = v1
        wt[64, c:c + 64] = bfg
        wt[65, c:c + 64] = f64(gcn_b)
        wt[32 + t, c + 64:c + 128] = p0
        wt[32 + t + 1, c + 64:c + 128] = p1
        wt[65, c + 64:c + 128] = bgv

    pack16 = np.zeros((128, _F16), dtype=np.float16)

    def put16(nm, arr):
        a = np.asarray(arr, dtype=np.float16)
        pack16[:a.shape[0], _OFF16[nm]:_OFF16[nm] + a.shape[1]] = a

    put16("at", part(AT, NT))
    put16("wt", wt)
    put16("skt", f64(skip_w).T)
    put16("e1t", part(f64(end1_w).T, SC // 128))
    put16("e2t", part(f64(end2_w).T, EC // 128))

    pack32 = np.zeros((128, _F32), dtype=np.float32)

    def put32(nm, arr):
        a = np.asarray(arr, dtype=np.float32)
        pack32[:a.shape[0], _OFF32[nm]:_OFF32[nm] + a.shape[1]] = a

    put32("skb", f64(skip_b).reshape(SC // 128, 128).T)
    put32("e1b", f64(end1_b).reshape(EC // 128, 128).T)
    put32("e2b", np.asarray(end2_b).reshape(P, 1))

    in_maps = []
    for b in range(B):
        pk = pack16.copy()
        xb = x[b]                                      # [T, N]
        pk[:, _OFF16["xT"]:_OFF16["xT"] + NT * T] = part(xb.T, NT)
        xnr = np.zeros((34, N))
        xnr[0:32] = xb
        xnr[32] = rowsum
        xnr[33] = 1.0
        pk[0:34, _OFF16["xnr"]:_OFF16["xnr"] + N] = xnr.astype(np.float16)
        in_maps.append({"H": pk, "F": pack32})

    nc = _get_nc()
    _NC_CACHE["in_maps"] = in_maps
    res = run_bass_kernel_spmd(nc, in_maps, list(range(NCORES)))
    out = np.stack([res.results[i]["out"] for i in range(B)])
    return out.astype(np.float32)                       # [B, P, N]


# revision 41
# speedup vs baseline: 1.1558x; 1.0579x over previous
"""GraphWaveNet block kernel for 8 Trainium2 NeuronCores (v2).

Math (reference reduced; res_w branch is dead code):
  A = gcn_norm adjacency [N,N]; xa[m,t] = sum_n A[m,n] x[t,n]
  fg[o,m,t] = v0[o] xa[m,t] + v1[o] xa[m,t+1] + rowsum[m] bfg[o] + gcn_b[o]
  g [o,n,t] = p0[o] x[t,n]  + p1[o] x[t+1,n]  + bg[o]
  hg = tanh(fg) * sigmoid(g)
  out = end2 @ mean_t relu(end1 @ relu(skip @ hg + skb) + e1b) / TO + e2b

Design vs v1 baseline:
  - fg AND g for one t come out of a single K=66 fp16 matmul, channel-major
    (contract over a stacked rhs R = [xaT(32); x(32); rowsum; ones]), so no
    PE transposes and no broadcast elementwise ops.
  - all big GEMMs run in fp16 (PE full rate, half the SBUF read power of
    fp32r -> HAM stays at 2.4 GHz), PSUM accumulation stays fp32.
  - loop software-skewed: fg at t, skip at t-1, end1 at t-2, so the PE
    queue never waits on Act/DVE within a step.
  - the time-mean is folded into the end2 matmul: e2 @ r1_t accumulates
    in one PSUM bank across all 31 steps (one long accumulation group),
    so no separate mean pass exists at all.
  - relu+bias on Act (bias operand) for the skip conv and on DVE
    (tensor_scalar add+max) for end1; hg = tanh*sigmoid on GpSimd.

Sharding: data-parallel over batch, 1 batch element per core (B=8).
"""

import numpy as np

from concourse import bacc
from concourse import mybir
from concourse.bass_utils import run_bass_kernel_spmd
from concourse.tile import TileContext

FP = mybir.dt.float32
F16 = mybir.dt.float16

B, T, N, E = 8, 32, 512, 8192
TO = T - 1          # output time steps
RC = DC = 64
SC, EC, P = 256, 512, 12
NCORES = 8
NT = N // 128       # node tiles
KR = 66             # stacked-rhs contraction: 32 xaT + 32 x + rowsum + ones

# fp16 packed-constant layout: name -> free-dim width of the [128, w] segment
_SEGS16 = [
    ("xT", NT * T),            # lhsT tiles for phase A  [128 n, (kt, t)]
    ("at", NT * N),            # A^T tiles (rhs phase A) [128 n, (kt, m)]
    ("xnr", N),                # rows 0-31 x natural; row 32 rowsum; row 33 ones
    ("wt", TO * 128),          # rows 0-65: per-t fg/g lhsT matrices
    ("skt", SC),               # rows 0-63: skip_w^T
    ("e1t", (SC // 128) * EC),  # end1_w^T tiles [128, (kj, m)]
    ("e2t", (EC // 128) * 128),  # end2_w^T tiles, M padded to 128
                               # (keeps PE tile_size 128x128, no mode switch)
]
_OFF16 = {}
_F16 = 0
for _nm, _w in _SEGS16:
    _OFF16[_nm] = _F16
    _F16 += _w

# fp32 packed-constant layout
_SEGS32 = [
    ("skb", SC // 128),
    ("e1b", EC // 128),
    ("e2b", 1),
]
_OFF32 = {}
_F32 = 0
for _nm, _w in _SEGS32:
    _OFF32[_nm] = _F32
    _F32 += _w


def _gcn_adj(edge_index, edge_weight, n):
    ei = np.asarray(edge_index)
    ew = np.asarray(edge_weight, dtype=np.float64)
    ar = np.arange(n)
    row = np.concatenate([ei[0], ar])
    col = np.concatenate([ei[1], ar])
    w = np.concatenate([ew, np.ones(n)])
    deg = np.zeros(n)
    np.add.at(deg, col, w)
    dis = np.where(deg > 0, 1.0 / np.sqrt(np.maximum(deg, 1e-300)), 0.0)
    norm = dis[row] * w * dis[col]
    A = np.zeros((n, n))
    np.add.at(A, (col, row), norm)
    return A  # A[tgt, src]


def _build_nc():
    nc = bacc.Bacc()
    d_h = nc.declare_dram_parameter("H", [128, _F16], F16, isOutput=False)
    d_f = nc.declare_dram_parameter("F", [128, _F32], FP, isOutput=False)
    d_out = nc.declare_dram_parameter("out", [P, N], FP, isOutput=True)

    AluOp = mybir.AluOpType
    Act = mybir.ActivationFunctionType

    o_xT = _OFF16["xT"]
    o_at = _OFF16["at"]
    o_xnr = _OFF16["xnr"]
    o_wt = _OFF16["wt"]
    o_skt = _OFF16["skt"]
    o_e1t = _OFF16["e1t"]
    o_e2t = _OFF16["e2t"]
    o_skb = _OFF32["skb"]
    o_e1b = _OFF32["e1b"]
    o_e2b = _OFF32["e2b"]

    with TileContext(nc) as tc:
        with (
            tc.tile_pool(name="consts", bufs=1) as consts,
            tc.tile_pool(name="work", bufs=2) as work,
            tc.tile_pool(name="ps", bufs=1, space="PSUM") as ps,
        ):
            ct = consts.tile([128, _F16], F16)
            cf = consts.tile([128, _F32], FP)
            R = consts.tile([KR, N], F16)
            # DMAs issued from different engines so the descriptor writes
            # don't serialize on one queue; transfers trimmed to the
            # partition rows actually used (xnr region is only read via
            # the R DMA).
            o_at1 = o_at + N
            nc.sync.dma_start(out=ct[:, :o_at1], in_=d_h[:, :o_at1])
            nc.sync.dma_start(out=ct[:, o_at1:o_xnr], in_=d_h[:, o_at1:o_xnr])
            nc.gpsimd.dma_start(
                out=ct[0:KR, o_wt:o_skt], in_=d_h[0:KR, o_wt:o_skt])
            nc.gpsimd.dma_start(
                out=ct[0:65, o_skt:o_e1t], in_=d_h[0:65, o_skt:o_e1t])
            # stacked rhs R: rows 0-31 xaT (computed), 32-63 x, 64 rowsum,
            # 65 ones
            nc.scalar.dma_start(out=R[32:KR], in_=d_h[0:34, o_xnr:o_xnr + N])
            nc.scalar.dma_start(out=ct[:, o_e1t:], in_=d_h[:, o_e1t:])
            nc.scalar.dma_start(out=cf[:], in_=d_f[:])

            # prime the one activation table set during the DMA wait
            prime = consts.tile([1, 1], FP)
            nc.vector.memset(prime[:], 0.0)
            nc.scalar.activation(prime[:], prime[:], Act.Sigmoid)

            # ---- phase A: xaT[t, m] = sum_n x[t, n] AT[n, m] ----
            xaps = ps.tile([32, N], FP, tag="pa", bufs=1)
            for kt in range(NT):
                nc.tensor.matmul(
                    xaps[:],
                    ct[:, o_xT + kt * T: o_xT + (kt + 1) * T],
                    ct[:, o_at + kt * N: o_at + (kt + 1) * N],
                    start=(kt == 0),
                    stop=(kt == NT - 1),
                )
            nc.scalar.copy(out=R[0:32], in_=xaps[:])

            # end2 accumulator: one PSUM bank, one accumulation group
            # spanning all (t, kj) — folds the time-mean into the matmul
            e2acc = ps.tile([128, N], FP, tag="pa", bufs=1)

            hgs = {}
            rsss = {}
            r1s = {}

            FGLEAD = 1  # fg stage runs this many t ahead of the skip stage

            def fg_stage(t):
                fgps = ps.tile([128, N], FP, tag="mm", bufs=7, name="fgps")
                nc.tensor.matmul(
                    fgps[:],
                    ct[0:KR, o_wt + t * 128: o_wt + (t + 1) * 128],
                    R[:],
                    start=True,
                    stop=True,
                )
                tnh = work.tile([64, N], F16, tag="tnh", bufs=FGLEAD + 2)
                sgm = work.tile([64, N], F16, tag="sgm", bufs=FGLEAD + 2)
                nc.scalar.activation(tnh[:], fgps[0:64], Act.Tanh)
                nc.scalar.activation(sgm[:], fgps[64:128], Act.Sigmoid)
                hg = work.tile([65, N], F16, tag="hg", bufs=FGLEAD + 2)
                nc.gpsimd.tensor_tensor(hg[0:64], tnh[:], sgm[:], AluOp.mult)
                nc.gpsimd.memset(hg[64:65], 0.0)
                hgs[t] = hg

            # warmup: run the fg stage ahead so the PE has dense work while
            # the Act->GpSimd hg chain fills
            for t in range(min(FGLEAD, TO)):
                fg_stage(t)

            # skewed main loop: fg at s+FGLEAD-1, skip at s-1, end1 at s-2,
            # end2 at s-3
            for s in range(1, TO + 3):
                t = s + FGLEAD - 1
                if t < TO:
                    fg_stage(t)

                tk = s - 1
                if 0 <= tk < TO:
                    hg = hgs.pop(tk)
                    sks = []
                    for sj in range(SC // 128):
                        skps = ps.tile([128, N], FP, tag="mm", bufs=7)
                        nc.tensor.matmul(
                            skps[:],
                            ct[0:65, o_skt + sj * 128: o_skt + (sj + 1) * 128],
                            hg[:],
                            start=True,
                            stop=True,
                        )
                        sks.append(skps)
                    rss = []
                    for sj in range(SC // 128):
                        rs = work.tile([128, N], F16, tag=f"rss{sj}", bufs=4)
                        bias = cf[:, o_skb + sj: o_skb + sj + 1]
                        nc.scalar.activation(
                            rs[:], sks[sj][:], Act.Relu,
                            bias=bias, scale=1.0,
                        )
                        rss.append(rs)
                    rsss[tk] = rss

                te = s - 2
                t2 = s - 3

                def e1_mms(mj, rss):
                    e1ps = ps.tile([128, N], FP, tag="mm", bufs=7,
                                   name="e1ps")
                    for kj in range(SC // 128):
                        nc.tensor.matmul(
                            e1ps[:],
                            ct[:, o_e1t + kj * EC + mj * 128:
                               o_e1t + kj * EC + (mj + 1) * 128],
                            rss[kj][:],
                            start=(kj == 0),
                            stop=(kj == SC // 128 - 1),
                        )
                    return e1ps

                e1list = []
                if 0 <= te < TO:
                    rss = rsss.pop(te)
                    for mj in range(EC // 128):
                        e1list.append(e1_mms(mj, rss))

                # end2 accumulation for t = s-3 (one group over all t, kj)
                if 0 <= t2 < TO:
                    r1l = r1s.pop(t2)
                    for kj in range(EC // 128):
                        nc.tensor.matmul(
                            e2acc[:],
                            ct[:, o_e2t + kj * 128: o_e2t + kj * 128 + 128],
                            r1l[kj][:],
                            start=(t2 == 0 and kj == 0),
                            stop=(t2 == TO - 1 and kj == EC // 128 - 1),
                        )

                if 0 <= te < TO:
                    r1l = []
                    for mj in range(EC // 128):
                        r1 = work.tile([128, N], F16, tag=f"r1_{mj}", bufs=3)
                        bias = cf[:, o_e1b + mj: o_e1b + mj + 1]
                        nc.vector.tensor_scalar(
                            r1[:], e1list[mj][:], bias, 0.0,
                            AluOp.add, AluOp.max,
                        )
                        r1l.append(r1)
                    r1s[te] = r1l

            # ---- scale 1/TO + bias on the accumulated end2 sum ----
            outsb = consts.tile([P, N], FP)
            nc.scalar.activation(
                outsb[:], e2acc[0:P, :], Act.Identity,
                bias=cf[0:P, o_e2b: o_e2b + 1], scale=1.0 / TO,
            )
            nc.sync.dma_start(out=d_out[:], in_=outsb[:])

    return nc


_NC_CACHE = {}


def _get_nc():
    if "nc" not in _NC_CACHE:
        nc = _build_nc()
        nc.finalize()
        _NC_CACHE["nc"] = nc
    return _NC_CACHE["nc"]


def kernel(x, edge_index, edge_weight, start_w, start_b, filt_w, filt_b,
           gate_w, gate_b, gcn_w, gcn_b, res_w, res_b, skip_w, skip_b,
           end1_w, end1_b, end2_w, end2_b, **_unused):
    x = np.asarray(x, dtype=np.float64)
    A = _gcn_adj(edge_index, edge_weight, N)          # float64 [tgt, src]
    rowsum = A.sum(axis=1)

    f64 = lambda a: np.asarray(a, dtype=np.float64)  # noqa: E731
    s = f64(start_w)[:, 0]
    sb = f64(start_b)
    fw, gw = f64(filt_w), f64(gate_w)
    gcn = f64(gcn_w)
    v0 = gcn @ (fw[:, :, 0] @ s)
    v1 = gcn @ (fw[:, :, 1] @ s)
    bfg = gcn @ ((fw[:, :, 0] + fw[:, :, 1]) @ sb + f64(filt_b))
    p0 = gw[:, :, 0] @ s
    p1 = gw[:, :, 1] @ s
    bgv = (gw[:, :, 0] + gw[:, :, 1]) @ sb + f64(gate_b)

    def part(a, ktiles):  # [(ktiles*128), M] -> [128, ktiles*M]
        a = np.asarray(a)
        return a.reshape(ktiles, 128, -1).transpose(1, 0, 2).reshape(128, -1)

    AT = np.ascontiguousarray(A.T)                     # [src n, tgt m]

    # per-t fg/g lhsT matrices, stacked along free dim
    wt = np.zeros((KR, TO * 128))
    for t in range(TO):
        c = t * 128
        wt[t, c:c + 64] = v0
        wt[t + 1, c:c + 64] = v1
        wt[64, c:c + 64] = bfg
        wt[65, c:c + 64] = f64(gcn_b)
        wt[32 + t, c + 64:c + 128] = p0
        wt[32 + t + 1, c + 64:c + 128] = p1
        wt[65, c + 64:c + 128] = bgv

    pack16 = np.zeros((128, _F16), dtype=np.float16)

    def put16(nm, arr):
        a = np.asarray(arr, dtype=np.float16)
        pack16[:a.shape[0], _OFF16[nm]:_OFF16[nm] + a.shape[1]] = a

    put16("at", part(AT, NT))
    put16("wt", wt)
    put16("skt", f64(skip_w).T)
    put16("e1t", part(f64(end1_w).T, SC // 128))
    e2wT = f64(end2_w).T                               # [EC, P]
    e2tp = np.zeros((128, (EC // 128) * 128))
    for kj in range(EC // 128):
        e2tp[:, kj * 128: kj * 128 + P] = e2wT[kj * 128:(kj + 1) * 128, :]
    put16("e2t", e2tp)

    pack32 = np.zeros((128, _F32), dtype=np.float32)

    def put32(nm, arr):
        a = np.asarray(arr, dtype=np.float32)
        pack32[:a.shape[0], _OFF32[nm]:_OFF32[nm] + a.shape[1]] = a

    put32("skb", f64(skip_b).reshape(SC // 128, 128).T)
    put32("e1b", f64(end1_b).reshape(EC // 128, 128).T)
    put32("e2b", np.asarray(end2_b).reshape(P, 1))

    in_maps = []
    for b in range(B):
        pk = pack16.copy()
        xb = x[b]                                      # [T, N]
        pk[:, _OFF16["xT"]:_OFF16["xT"] + NT * T] = part(xb.T, NT)
        xnr = np.zeros((34, N))
        xnr[0:32] = xb
        xnr[32] = rowsum
        xnr[33] = 1.0
        pk[0:34, _OFF16["xnr"]:_OFF16["xnr"] + N] = xnr.astype(np.float16)
        in_maps.append({"H": pk, "F": pack32})

    nc = _get_nc()
    _NC_CACHE["in_maps"] = in_maps
    res = run_bass_kernel_spmd(nc, in_maps, list(range(NCORES)))
    out = np.stack([res.results[i]["out"] for i in range(B)])
    return out.astype(np.float32)                       # [B, P, N]


# revision 42
# speedup vs baseline: 1.1678x; 1.0103x over previous
"""GraphWaveNet block kernel for 8 Trainium2 NeuronCores (v2).

Math (reference reduced; res_w branch is dead code):
  A = gcn_norm adjacency [N,N]; xa[m,t] = sum_n A[m,n] x[t,n]
  fg[o,m,t] = v0[o] xa[m,t] + v1[o] xa[m,t+1] + rowsum[m] bfg[o] + gcn_b[o]
  g [o,n,t] = p0[o] x[t,n]  + p1[o] x[t+1,n]  + bg[o]
  hg = tanh(fg) * sigmoid(g)
  out = end2 @ mean_t relu(end1 @ relu(skip @ hg + skb) + e1b) / TO + e2b

Design vs v1 baseline:
  - fg AND g for one t come out of a single K=66 fp16 matmul, channel-major
    (contract over a stacked rhs R = [xaT(32); x(32); rowsum; ones]), so no
    PE transposes and no broadcast elementwise ops.
  - all big GEMMs run in fp16 (PE full rate, half the SBUF read power of
    fp32r -> HAM stays at 2.4 GHz), PSUM accumulation stays fp32.
  - loop software-skewed: fg at t, skip at t-1, end1 at t-2, so the PE
    queue never waits on Act/DVE within a step.
  - the time-mean is folded into the end2 matmul: e2 @ r1_t accumulates
    in one PSUM bank across all 31 steps (one long accumulation group),
    so no separate mean pass exists at all.
  - relu+bias on Act (bias operand) for the skip conv and on DVE
    (tensor_scalar add+max) for end1; hg = tanh*sigmoid on GpSimd.

Sharding: data-parallel over batch, 1 batch element per core (B=8).
"""

import numpy as np

from concourse import bacc
from concourse import mybir
from concourse.bass_utils import run_bass_kernel_spmd
from concourse.tile import TileContext

FP = mybir.dt.float32
F16 = mybir.dt.float16

B, T, N, E = 8, 32, 512, 8192
TO = T - 1          # output time steps
RC = DC = 64
SC, EC, P = 256, 512, 12
NCORES = 8
NT = N // 128       # node tiles
KR = 66             # stacked-rhs contraction: 32 xaT + 32 x + rowsum + ones

# fp16 packed-constant layout: name -> free-dim width of the [128, w] segment
_SEGS16 = [
    ("xT", NT * T),            # lhsT tiles for phase A  [128 n, (kt, t)]
    ("at", NT * N),            # A^T tiles (rhs phase A) [128 n, (kt, m)]
    ("xnr", N),                # rows 0-31 x natural; row 32 rowsum; row 33 ones
    ("wt", TO * 128),          # rows 0-65: per-t fg/g lhsT matrices
    ("skt", SC),               # rows 0-63: skip_w^T
    ("e1t", (SC // 128) * EC),  # end1_w^T tiles [128, (kj, m)]
    ("e2t", (EC // 128) * 128),  # end2_w^T tiles, M padded to 128
                               # (keeps PE tile_size 128x128, no mode switch)
]
_OFF16 = {}
_F16 = 0
for _nm, _w in _SEGS16:
    _OFF16[_nm] = _F16
    _F16 += _w

# fp32 packed-constant layout
_SEGS32 = [
    ("skb", SC // 128),
    ("e1b", EC // 128),
    ("e2b", 1),
]
_OFF32 = {}
_F32 = 0
for _nm, _w in _SEGS32:
    _OFF32[_nm] = _F32
    _F32 += _w


def _gcn_adj(edge_index, edge_weight, n):
    ei = np.asarray(edge_index)
    ew = np.asarray(edge_weight, dtype=np.float64)
    ar = np.arange(n)
    row = np.concatenate([ei[0], ar])
    col = np.concatenate([ei[1], ar])
    w = np.concatenate([ew, np.ones(n)])
    deg = np.zeros(n)
    np.add.at(deg, col, w)
    dis = np.where(deg > 0, 1.0 / np.sqrt(np.maximum(deg, 1e-300)), 0.0)
    norm = dis[row] * w * dis[col]
    A = np.zeros((n, n))
    np.add.at(A, (col, row), norm)
    return A  # A[tgt, src]


def _build_nc():
    nc = bacc.Bacc()
    d_h = nc.declare_dram_parameter("H", [128, _F16], F16, isOutput=False)
    d_f = nc.declare_dram_parameter("F", [128, _F32], FP, isOutput=False)
    d_out = nc.declare_dram_parameter("out", [P, N], FP, isOutput=True)

    AluOp = mybir.AluOpType
    Act = mybir.ActivationFunctionType

    o_xT = _OFF16["xT"]
    o_at = _OFF16["at"]
    o_xnr = _OFF16["xnr"]
    o_wt = _OFF16["wt"]
    o_skt = _OFF16["skt"]
    o_e1t = _OFF16["e1t"]
    o_e2t = _OFF16["e2t"]
    o_skb = _OFF32["skb"]
    o_e1b = _OFF32["e1b"]
    o_e2b = _OFF32["e2b"]

    with TileContext(nc) as tc:
        with (
            tc.tile_pool(name="consts", bufs=1) as consts,
            tc.tile_pool(name="work", bufs=2) as work,
            tc.tile_pool(name="ps", bufs=1, space="PSUM") as ps,
        ):
            ct = consts.tile([128, _F16], F16)
            cf = consts.tile([128, _F32], FP)
            R = consts.tile([KR, N], F16)
            # DMAs issued from different engines so the descriptor writes
            # don't serialize on one queue; transfers trimmed to the
            # partition rows actually used (xnr region is only read via
            # the R DMA).
            o_at1 = o_at + N
            nc.sync.dma_start(out=ct[:, :o_at1], in_=d_h[:, :o_at1])
            nc.sync.dma_start(out=ct[:, o_at1:o_xnr], in_=d_h[:, o_at1:o_xnr])
            nc.gpsimd.dma_start(
                out=ct[0:KR, o_wt:o_skt], in_=d_h[0:KR, o_wt:o_skt])
            nc.gpsimd.dma_start(
                out=ct[0:65, o_skt:o_e1t], in_=d_h[0:65, o_skt:o_e1t])
            # stacked rhs R: rows 0-31 xaT (computed), 32-63 x, 64 rowsum,
            # 65 ones
            nc.gpsimd.dma_start(out=R[32:KR], in_=d_h[0:34, o_xnr:o_xnr + N])
            nc.gpsimd.dma_start(out=ct[:, o_e1t:], in_=d_h[:, o_e1t:])
            nc.gpsimd.dma_start(out=cf[:], in_=d_f[:])

            # prime the one activation table set during the DMA wait
            prime = consts.tile([1, 1], FP)
            nc.vector.memset(prime[:], 0.0)
            nc.scalar.activation(prime[:], prime[:], Act.Sigmoid)

            # ---- phase A: xaT[t, m] = sum_n x[t, n] AT[n, m] ----
            xaps = ps.tile([32, N], FP, tag="pa", bufs=1)
            for kt in range(NT):
                nc.tensor.matmul(
                    xaps[:],
                    ct[:, o_xT + kt * T: o_xT + (kt + 1) * T],
                    ct[:, o_at + kt * N: o_at + (kt + 1) * N],
                    start=(kt == 0),
                    stop=(kt == NT - 1),
                )
            nc.scalar.copy(out=R[0:32], in_=xaps[:])

            # end2 accumulator: one PSUM bank, one accumulation group
            # spanning all (t, kj) — folds the time-mean into the matmul
            e2acc = ps.tile([128, N], FP, tag="pa", bufs=1)

            hgs = {}
            rsss = {}
            r1s = {}

            FGLEAD = 1  # fg stage runs this many t ahead of the skip stage

            def fg_stage(t):
                fgps = ps.tile([128, N], FP, tag="mm", bufs=7, name="fgps")
                nc.tensor.matmul(
                    fgps[:],
                    ct[0:KR, o_wt + t * 128: o_wt + (t + 1) * 128],
                    R[:],
                    start=True,
                    stop=True,
                )
                tnh = work.tile([64, N], F16, tag="tnh", bufs=FGLEAD + 2)
                sgm = work.tile([64, N], F16, tag="sgm", bufs=FGLEAD + 2)
                nc.scalar.activation(tnh[:], fgps[0:64], Act.Tanh)
                nc.scalar.activation(sgm[:], fgps[64:128], Act.Sigmoid)
                hg = work.tile([65, N], F16, tag="hg", bufs=FGLEAD + 2)
                nc.gpsimd.memset(hg[64:65], 0.0)
                nc.gpsimd.tensor_tensor(hg[0:64], tnh[:], sgm[:], AluOp.mult)
                hgs[t] = hg

            # warmup: run the fg stage ahead so the PE has dense work while
            # the Act->GpSimd hg chain fills
            for t in range(min(FGLEAD, TO)):
                fg_stage(t)

            # skewed main loop: fg at s+FGLEAD-1, skip at s-1, end1 at s-2,
            # end2 at s-3
            for s in range(1, TO + 3):
                t = s + FGLEAD - 1
                if t < TO:
                    fg_stage(t)

                tk = s - 1
                if 0 <= tk < TO:
                    hg = hgs.pop(tk)
                    sks = []
                    for sj in range(SC // 128):
                        skps = ps.tile([128, N], FP, tag="mm", bufs=7)
                        nc.tensor.matmul(
                            skps[:],
                            ct[0:65, o_skt + sj * 128: o_skt + (sj + 1) * 128],
                            hg[:],
                            start=True,
                            stop=True,
                        )
                        sks.append(skps)
                    rss = []
                    for sj in range(SC // 128):
                        rs = work.tile([128, N], F16, tag=f"rss{sj}", bufs=4)
                        bias = cf[:, o_skb + sj: o_skb + sj + 1]
                        nc.scalar.activation(
                            rs[:], sks[sj][:], Act.Relu,
                            bias=bias, scale=1.0,
                        )
                        rss.append(rs)
                    rsss[tk] = rss

                te = s - 2
                t2 = s - 3

                def e1_mms(mj, rss):
                    e1ps = ps.tile([128, N], FP, tag="mm", bufs=7,
                                   name="e1ps")
                    for kj in range(SC // 128):
                        nc.tensor.matmul(
                            e1ps[:],
                            ct[:, o_e1t + kj * EC + mj * 128:
                               o_e1t + kj * EC + (mj + 1) * 128],
                            rss[kj][:],
                            start=(kj == 0),
                            stop=(kj == SC // 128 - 1),
                        )
                    return e1ps

                e1list = []
                if 0 <= te < TO:
                    rss = rsss.pop(te)
                    for mj in range(EC // 128):
                        e1list.append(e1_mms(mj, rss))

                # end2 accumulation for t = s-3 (one group over all t, kj)
                if 0 <= t2 < TO:
                    r1l = r1s.pop(t2)
                    for kj in range(EC // 128):
                        nc.tensor.matmul(
                            e2acc[:],
                            ct[:, o_e2t + kj * 128: o_e2t + kj * 128 + 128],
                            r1l[kj][:],
                            start=(t2 == 0 and kj == 0),
                            stop=(t2 == TO - 1 and kj == EC // 128 - 1),
                        )

                if 0 <= te < TO:
                    r1l = []
                    for mj in range(EC // 128):
                        r1 = work.tile([128, N], F16, tag=f"r1_{mj}", bufs=3)
                        bias = cf[:, o_e1b + mj: o_e1b + mj + 1]
                        nc.vector.tensor_scalar(
                            r1[:], e1list[mj][:], bias, 0.0,
                            AluOp.add, AluOp.max,
                        )
                        r1l.append(r1)
                    r1s[te] = r1l

            # ---- scale 1/TO + bias on the accumulated end2 sum ----
            outsb = consts.tile([P, N], FP)
            nc.scalar.activation(
                outsb[:], e2acc[0:P, :], Act.Identity,
                bias=cf[0:P, o_e2b: o_e2b + 1], scale=1.0 / TO,
            )
            nc.sync.dma_start(out=d_out[:], in_=outsb[:])

    return nc


_NC_CACHE = {}


def _get_nc():
    if "nc" not in _NC_CACHE:
        nc = _build_nc()
        nc.finalize()
        _NC_CACHE["nc"] = nc
    return _NC_CACHE["nc"]


def kernel(x, edge_index, edge_weight, start_w, start_b, filt_w, filt_b,
           gate_w, gate_b, gcn_w, gcn_b, res_w, res_b, skip_w, skip_b,
           end1_w, end1_b, end2_w, end2_b, **_unused):
    x = np.asarray(x, dtype=np.float64)
    A = _gcn_adj(edge_index, edge_weight, N)          # float64 [tgt, src]
    rowsum = A.sum(axis=1)

    f64 = lambda a: np.asarray(a, dtype=np.float64)  # noqa: E731
    s = f64(start_w)[:, 0]
    sb = f64(start_b)
    fw, gw = f64(filt_w), f64(gate_w)
    gcn = f64(gcn_w)
    v0 = gcn @ (fw[:, :, 0] @ s)
    v1 = gcn @ (fw[:, :, 1] @ s)
    bfg = gcn @ ((fw[:, :, 0] + fw[:, :, 1]) @ sb + f64(filt_b))
    p0 = gw[:, :, 0] @ s
    p1 = gw[:, :, 1] @ s
    bgv = (gw[:, :, 0] + gw[:, :, 1]) @ sb + f64(gate_b)

    def part(a, ktiles):  # [(ktiles*128), M] -> [128, ktiles*M]
        a = np.asarray(a)
        return a.reshape(ktiles, 128, -1).transpose(1, 0, 2).reshape(128, -1)

    AT = np.ascontiguousarray(A.T)                     # [src n, tgt m]

    # per-t fg/g lhsT matrices, stacked along free dim
    wt = np.zeros((KR, TO * 128))
    for t in range(TO):
        c = t * 128
        wt[t, c:c + 64] = v0
        wt[t + 1, c:c + 64] = v1
        wt[64, c:c + 64] = bfg
        wt[65, c:c + 64] = f64(gcn_b)
        wt[32 + t, c + 64:c + 128] = p0
        wt[32 + t + 1, c + 64:c + 128] = p1
        wt[65, c + 64:c + 128] = bgv

    pack16 = np.zeros((128, _F16), dtype=np.float16)

    def put16(nm, arr):
        a = np.asarray(arr, dtype=np.float16)
        pack16[:a.shape[0], _OFF16[nm]:_OFF16[nm] + a.shape[1]] = a

    put16("at", part(AT, NT))
    put16("wt", wt)
    put16("skt", f64(skip_w).T)
    put16("e1t", part(f64(end1_w).T, SC // 128))
    e2wT = f64(end2_w).T                               # [EC, P]
    e2tp = np.zeros((128, (EC // 128) * 128))
    for kj in range(EC // 128):
        e2tp[:, kj * 128: kj * 128 + P] = e2wT[kj * 128:(kj + 1) * 128, :]
    put16("e2t", e2tp)

    pack32 = np.zeros((128, _F32), dtype=np.float32)

    def put32(nm, arr):
        a = np.asarray(arr, dtype=np.float32)
        pack32[:a.shape[0], _OFF32[nm]:_OFF32[nm] + a.shape[1]] = a

    put32("skb", f64(skip_b).reshape(SC // 128, 128).T)
    put32("e1b", f64(end1_b).reshape(EC // 128, 128).T)
    put32("e2b", np.asarray(end2_b).reshape(P, 1))

    in_maps = []
    for b in range(B):
        pk = pack16.copy()
        xb = x[b]                                      # [T, N]
        pk[:, _OFF16["xT"]:_OFF16["xT"] + NT * T] = part(xb.T, NT)
        xnr = np.zeros((34, N))
        xnr[0:32] = xb
        xnr[32] = rowsum
        xnr[33] = 1.0
        pk[0:34, _OFF16["xnr"]:_OFF16["xnr"] + N] = xnr.astype(np.float16)
        in_maps.append({"H": pk, "F": pack32})

    nc = _get_nc()
    _NC_CACHE["in_maps"] = in_maps
    res = run_bass_kernel_spmd(nc, in_maps, list(range(NCORES)))
    out = np.stack([res.results[i]["out"] for i in range(B)])
    return out.astype(np.float32)                       # [B, P, N]


# revision 43
# speedup vs baseline: 1.1801x; 1.0106x over previous
"""GraphWaveNet block kernel for 8 Trainium2 NeuronCores (v2).

Math (reference reduced; res_w branch is dead code):
  A = gcn_norm adjacency [N,N]; xa[m,t] = sum_n A[m,n] x[t,n]
  fg[o,m,t] = v0[o] xa[m,t] + v1[o] xa[m,t+1] + rowsum[m] bfg[o] + gcn_b[o]
  g [o,n,t] = p0[o] x[t,n]  + p1[o] x[t+1,n]  + bg[o]
  hg = tanh(fg) * sigmoid(g)
  out = end2 @ mean_t relu(end1 @ relu(skip @ hg + skb) + e1b) / TO + e2b

Design vs v1 baseline:
  - fg AND g for one t come out of a single K=66 fp16 matmul, channel-major
    (contract over a stacked rhs R = [xaT(32); x(32); rowsum; ones]), so no
    PE transposes and no broadcast elementwise ops.
  - all big GEMMs run in fp16 (PE full rate, half the SBUF read power of
    fp32r -> HAM stays at 2.4 GHz), PSUM accumulation stays fp32.
  - loop software-skewed: fg at t, skip at t-1, end1 at t-2, so the PE
    queue never waits on Act/DVE within a step.
  - the time-mean is folded into the end2 matmul: e2 @ r1_t accumulates
    in one PSUM bank across all 31 steps (one long accumulation group),
    so no separate mean pass exists at all.
  - relu+bias on Act (bias operand) for the skip conv and on DVE
    (tensor_scalar add+max) for end1; hg = tanh*sigmoid on GpSimd.

Sharding: data-parallel over batch, 1 batch element per core (B=8).
"""

import numpy as np

from concourse import bacc
from concourse import mybir
from concourse.bass_utils import run_bass_kernel_spmd
from concourse.tile import TileContext

FP = mybir.dt.float32
F16 = mybir.dt.float16

B, T, N, E = 8, 32, 512, 8192
TO = T - 1          # output time steps
RC = DC = 64
SC, EC, P = 256, 512, 12
NCORES = 8
NT = N // 128       # node tiles
KR = 66             # stacked-rhs contraction: 32 xaT + 32 x + rowsum + ones

# fp16 packed-constant layout: name -> free-dim width of the [128, w] segment
_SEGS16 = [
    ("xT", NT * T),            # lhsT tiles for phase A  [128 n, (kt, t)]
    ("at", NT * N),            # A^T tiles (rhs phase A) [128 n, (kt, m)]
    ("xnr", N),                # rows 0-31 x natural; row 32 rowsum; row 33 ones
    ("wt", TO * 128),          # rows 0-65: per-t fg/g lhsT matrices
    ("skt", SC),               # rows 0-63: skip_w^T
    ("e1t", (SC // 128) * EC),  # end1_w^T tiles [128, (kj, m)]
    ("e2t", (EC // 128) * 128),  # end2_w^T tiles, M padded to 128
                               # (keeps PE tile_size 128x128, no mode switch)
]
_OFF16 = {}
_F16 = 0
for _nm, _w in _SEGS16:
    _OFF16[_nm] = _F16
    _F16 += _w

# fp32 packed-constant layout
_SEGS32 = [
    ("skb", SC // 128),
    ("e1b", EC // 128),
    ("e2b", 1),
]
_OFF32 = {}
_F32 = 0
for _nm, _w in _SEGS32:
    _OFF32[_nm] = _F32
    _F32 += _w


def _gcn_adj(edge_index, edge_weight, n):
    ei = np.asarray(edge_index)
    ew = np.asarray(edge_weight, dtype=np.float64)
    ar = np.arange(n)
    row = np.concatenate([ei[0], ar])
    col = np.concatenate([ei[1], ar])
    w = np.concatenate([ew, np.ones(n)])
    deg = np.zeros(n)
    np.add.at(deg, col, w)
    dis = np.where(deg > 0, 1.0 / np.sqrt(np.maximum(deg, 1e-300)), 0.0)
    norm = dis[row] * w * dis[col]
    A = np.zeros((n, n))
    np.add.at(A, (col, row), norm)
    return A  # A[tgt, src]


def _build_nc():
    nc = bacc.Bacc()
    d_h = nc.declare_dram_parameter("H", [128, _F16], F16, isOutput=False)
    d_f = nc.declare_dram_parameter("F", [128, _F32], FP, isOutput=False)
    d_out = nc.declare_dram_parameter("out", [P, N], FP, isOutput=True)

    AluOp = mybir.AluOpType
    Act = mybir.ActivationFunctionType

    o_xT = _OFF16["xT"]
    o_at = _OFF16["at"]
    o_xnr = _OFF16["xnr"]
    o_wt = _OFF16["wt"]
    o_skt = _OFF16["skt"]
    o_e1t = _OFF16["e1t"]
    o_e2t = _OFF16["e2t"]
    o_skb = _OFF32["skb"]
    o_e1b = _OFF32["e1b"]
    o_e2b = _OFF32["e2b"]

    with TileContext(nc) as tc:
        with (
            tc.tile_pool(name="consts", bufs=1) as consts,
            tc.tile_pool(name="work", bufs=2) as work,
            tc.tile_pool(name="ps", bufs=1, space="PSUM") as ps,
        ):
            ct = consts.tile([128, _F16], F16)
            cf = consts.tile([128, _F32], FP)
            R = consts.tile([KR, N], F16)
            # DMAs issued from different engines so the descriptor writes
            # don't serialize on one queue; transfers trimmed to the
            # partition rows actually used (xnr region is only read via
            # the R DMA).
            # priority order: phase-A inputs then fg weights serially on the
            # sync queue (the hw DMA engine round-robins across queues, so
            # early-needed data must not share a queue with bulk weights);
            # later-needed segments go on the gpsimd-issued queue.
            nc.sync.dma_start(out=ct[:, :o_xnr], in_=d_h[:, :o_xnr])
            nc.sync.dma_start(
                out=ct[0:KR, o_wt:o_skt], in_=d_h[0:KR, o_wt:o_skt])
            # stacked rhs R: rows 0-31 xaT (computed), 32-63 x, 64 rowsum,
            # 65 ones
            nc.gpsimd.dma_start(out=R[32:KR], in_=d_h[0:34, o_xnr:o_xnr + N])
            nc.gpsimd.dma_start(
                out=ct[0:65, o_skt:o_e1t], in_=d_h[0:65, o_skt:o_e1t])
            nc.gpsimd.dma_start(out=ct[:, o_e1t:], in_=d_h[:, o_e1t:])
            nc.gpsimd.dma_start(out=cf[:], in_=d_f[:])

            # prime the one activation table set during the DMA wait
            prime = consts.tile([1, 1], FP)
            nc.vector.memset(prime[:], 0.0)
            nc.scalar.activation(prime[:], prime[:], Act.Sigmoid)

            # ---- phase A: xaT[t, m] = sum_n x[t, n] AT[n, m] ----
            xaps = ps.tile([32, N], FP, tag="pa", bufs=1)
            for kt in range(NT):
                nc.tensor.matmul(
                    xaps[:],
                    ct[:, o_xT + kt * T: o_xT + (kt + 1) * T],
                    ct[:, o_at + kt * N: o_at + (kt + 1) * N],
                    start=(kt == 0),
                    stop=(kt == NT - 1),
                )
            nc.scalar.copy(out=R[0:32], in_=xaps[:])

            # end2 accumulator: one PSUM bank, one accumulation group
            # spanning all (t, kj) — folds the time-mean into the matmul
            e2acc = ps.tile([128, N], FP, tag="pa", bufs=1)

            hgs = {}
            rsss = {}
            r1s = {}

            FGLEAD = 1  # fg stage runs this many t ahead of the skip stage

            def fg_stage(t):
                fgps = ps.tile([128, N], FP, tag="mm", bufs=7, name="fgps")
                nc.tensor.matmul(
                    fgps[:],
                    ct[0:KR, o_wt + t * 128: o_wt + (t + 1) * 128],
                    R[:],
                    start=True,
                    stop=True,
                )
                tnh = work.tile([64, N], F16, tag="tnh", bufs=FGLEAD + 2)
                sgm = work.tile([64, N], F16, tag="sgm", bufs=FGLEAD + 2)
                nc.scalar.activation(tnh[:], fgps[0:64], Act.Tanh)
                nc.scalar.activation(sgm[:], fgps[64:128], Act.Sigmoid)
                hg = work.tile([65, N], F16, tag="hg", bufs=FGLEAD + 2)
                nc.gpsimd.memset(hg[64:65], 0.0)
                nc.gpsimd.tensor_tensor(hg[0:64], tnh[:], sgm[:], AluOp.mult)
                hgs[t] = hg

            # warmup: run the fg stage ahead so the PE has dense work while
            # the Act->GpSimd hg chain fills
            for t in range(min(FGLEAD, TO)):
                fg_stage(t)

            # skewed main loop: fg at s+FGLEAD-1, skip at s-1, end1 at s-2,
            # end2 at s-3
            for s in range(1, TO + 3):
                t = s + FGLEAD - 1
                if t < TO:
                    fg_stage(t)

                tk = s - 1
                if 0 <= tk < TO:
                    hg = hgs.pop(tk)
                    sks = []
                    for sj in range(SC // 128):
                        skps = ps.tile([128, N], FP, tag="mm", bufs=7)
                        nc.tensor.matmul(
                            skps[:],
                            ct[0:65, o_skt + sj * 128: o_skt + (sj + 1) * 128],
                            hg[:],
                            start=True,
                            stop=True,
                        )
                        sks.append(skps)
                    rss = []
                    for sj in range(SC // 128):
                        rs = work.tile([128, N], F16, tag=f"rss{sj}", bufs=4)
                        bias = cf[:, o_skb + sj: o_skb + sj + 1]
                        nc.scalar.activation(
                            rs[:], sks[sj][:], Act.Relu,
                            bias=bias, scale=1.0,
                        )
                        rss.append(rs)
                    rsss[tk] = rss

                te = s - 2
                t2 = s - 3

                def e1_mms(mj, rss):
                    e1ps = ps.tile([128, N], FP, tag="mm", bufs=7,
                                   name="e1ps")
                    for kj in range(SC // 128):
                        nc.tensor.matmul(
                            e1ps[:],
                            ct[:, o_e1t + kj * EC + mj * 128:
                               o_e1t + kj * EC + (mj + 1) * 128],
                            rss[kj][:],
                            start=(kj == 0),
                            stop=(kj == SC // 128 - 1),
                        )
                    return e1ps

                e1list = []
                if 0 <= te < TO:
                    rss = rsss.pop(te)
                    for mj in range(EC // 128):
                        e1list.append(e1_mms(mj, rss))

                # end2 accumulation for t = s-3 (one group over all t, kj)
                if 0 <= t2 < TO:
                    r1l = r1s.pop(t2)
                    for kj in range(EC // 128):
                        nc.tensor.matmul(
                            e2acc[:],
                            ct[:, o_e2t + kj * 128: o_e2t + kj * 128 + 128],
                            r1l[kj][:],
                            start=(t2 == 0 and kj == 0),
                            stop=(t2 == TO - 1 and kj == EC // 128 - 1),
                        )

                if 0 <= te < TO:
                    r1l = []
                    for mj in range(EC // 128):
                        r1 = work.tile([128, N], F16, tag=f"r1_{mj}", bufs=3)
                        bias = cf[:, o_e1b + mj: o_e1b + mj + 1]
                        nc.vector.tensor_scalar(
                            r1[:], e1list[mj][:], bias, 0.0,
                            AluOp.add, AluOp.max,
                        )
                        r1l.append(r1)
                    r1s[te] = r1l

            # ---- scale 1/TO + bias on the accumulated end2 sum ----
            outsb = consts.tile([P, N], FP)
            nc.scalar.activation(
                outsb[:], e2acc[0:P, :], Act.Identity,
                bias=cf[0:P, o_e2b: o_e2b + 1], scale=1.0 / TO,
            )
            nc.sync.dma_start(out=d_out[:], in_=outsb[:])

    return nc


_NC_CACHE = {}


def _get_nc():
    if "nc" not in _NC_CACHE:
        nc = _build_nc()
        nc.finalize()
        _NC_CACHE["nc"] = nc
    return _NC_CACHE["nc"]


def kernel(x, edge_index, edge_weight, start_w, start_b, filt_w, filt_b,
           gate_w, gate_b, gcn_w, gcn_b, res_w, res_b, skip_w, skip_b,
           end1_w, end1_b, end2_w, end2_b, **_unused):
    x = np.asarray(x, dtype=np.float64)
    A = _gcn_adj(edge_index, edge_weight, N)          # float64 [tgt, src]
    rowsum = A.sum(axis=1)

    f64 = lambda a: np.asarray(a, dtype=np.float64)  # noqa: E731
    s = f64(start_w)[:, 0]
    sb = f64(start_b)
    fw, gw = f64(filt_w), f64(gate_w)
    gcn = f64(gcn_w)
    v0 = gcn @ (fw[:, :, 0] @ s)
    v1 = gcn @ (fw[:, :, 1] @ s)
    bfg = gcn @ ((fw[:, :, 0] + fw[:, :, 1]) @ sb + f64(filt_b))
    p0 = gw[:, :, 0] @ s
    p1 = gw[:, :, 1] @ s
    bgv = (gw[:, :, 0] + gw[:, :, 1]) @ sb + f64(gate_b)

    def part(a, ktiles):  # [(ktiles*128), M] -> [128, ktiles*M]
        a = np.asarray(a)
        return a.reshape(ktiles, 128, -1).transpose(1, 0, 2).reshape(128, -1)

    AT = np.ascontiguousarray(A.T)                     # [src n, tgt m]

    # per-t fg/g lhsT matrices, stacked along free dim
    wt = np.zeros((KR, TO * 128))
    for t in range(TO):
        c = t * 128
        wt[t, c:c + 64] = v0
        wt[t + 1, c:c + 64] = v1
        wt[64, c:c + 64] = bfg
        wt[65, c:c + 64] = f64(gcn_b)
        wt[32 + t, c + 64:c + 128] = p0
        wt[32 + t + 1, c + 64:c + 128] = p1
        wt[65, c + 64:c + 128] = bgv

    pack16 = np.zeros((128, _F16), dtype=np.float16)

    def put16(nm, arr):
        a = np.asarray(arr, dtype=np.float16)
        pack16[:a.shape[0], _OFF16[nm]:_OFF16[nm] + a.shape[1]] = a

    put16("at", part(AT, NT))
    put16("wt", wt)
    put16("skt", f64(skip_w).T)
    put16("e1t", part(f64(end1_w).T, SC // 128))
    e2wT = f64(end2_w).T                               # [EC, P]
    e2tp = np.zeros((128, (EC // 128) * 128))
    for kj in range(EC // 128):
        e2tp[:, kj * 128: kj * 128 + P] = e2wT[kj * 128:(kj + 1) * 128, :]
    put16("e2t", e2tp)

    pack32 = np.zeros((128, _F32), dtype=np.float32)

    def put32(nm, arr):
        a = np.asarray(arr, dtype=np.float32)
        pack32[:a.shape[0], _OFF32[nm]:_OFF32[nm] + a.shape[1]] = a

    put32("skb", f64(skip_b).reshape(SC // 128, 128).T)
    put32("e1b", f64(end1_b).reshape(EC // 128, 128).T)
    put32("e2b", np.asarray(end2_b).reshape(P, 1))

    in_maps = []
    for b in range(B):
        pk = pack16.copy()
        xb = x[b]                                      # [T, N]
        pk[:, _OFF16["xT"]:_OFF16["xT"] + NT * T] = part(xb.T, NT)
        xnr = np.zeros((34, N))
        xnr[0:32] = xb
        xnr[32] = rowsum
        xnr[33] = 1.0
        pk[0:34, _OFF16["xnr"]:_OFF16["xnr"] + N] = xnr.astype(np.float16)
        in_maps.append({"H": pk, "F": pack32})

    nc = _get_nc()
    _NC_CACHE["in_maps"] = in_maps
    res = run_bass_kernel_spmd(nc, in_maps, list(range(NCORES)))
    out = np.stack([res.results[i]["out"] for i in range(B)])
    return out.astype(np.float32)                       # [B, P, N]
